# revision 2
# baseline (speedup 1.0000x reference)
"""Trainium2 Bass kernel for the packed-sequence CrossEntropy-style loss.

Problem (hardcoded shapes): scores [8, 1024, 32000] f32, target [8, 1024] int,
lengths [8] int (descending, lengths[0] = 1024).

reference math per batch row b:
    lp   = log_softmax(scores[b], axis=-1)                    # [T, V]
    lp_t = lp[t, target[t]]            (0 where t >= len)     # [T]
    p    = exp(lp_t)                   (1 where t >= len)
    props[0] = 0.5 ; props[t] = 0.3*props[t-1] + 0.7*p[t-1]
    soft = softmax(props over valid t) * len  (0 at invalid)
    partial_b = sum_t lp_t * soft
loss = -sum_b partial_b / sum_b len_b

Sharding: data-parallel over batch. Core b handles row b: streams its
[1024, 32000] f32 slab once from HBM (memory-bound), computes sum-exp with a
fused ACT exp+accumulate, gathers scores[t, target[t]] with an indirect DMA,
then runs the tiny serial tail (scan + ragged softmax) on a [1, 1024] row.
Host sums the 8 scalar partials (the "all-reduce") and divides by sum(len).

No max-subtraction in the log-sum-exp: inputs are N(0,1) so exp() is safely
in range and the fp32 sum of 32000 such terms is accurate to ~1e-7.
"""

import numpy as np
from contextlib import ExitStack

import concourse.bass as bass
import concourse.bacc as bacc
import concourse.tile as tile
from concourse import mybir
from concourse.bass_utils import run_bass_kernel_spmd
from concourse.masks import make_identity

B, T, V = 8, 1024, 32000
P = 128            # SBUF partitions
NBLK = T // P      # 8 blocks of 128 t-rows
NCHUNK = 8         # V chunks per block
VC = V // NCHUNK   # 4000 f32 = 16 KB contiguous per partition per tile
N_CORES = 8
LN07 = float(np.log(0.7))

F32 = mybir.dt.float32
I32 = mybir.dt.int32
Alu = mybir.AluOpType
Act = mybir.ActivationFunctionType


def _emit(ctx: ExitStack, tc: "tile.TileContext", scores, gidx, len_f, out):
    nc = tc.nc

    data = ctx.enter_context(tc.tile_pool(name="data", bufs=4))
    singles = ctx.enter_context(tc.tile_pool(name="singles", bufs=1))
    psum = ctx.enter_context(tc.tile_pool(name="psum", bufs=1, space="PSUM"))

    # flat [T*V, 1] view of scores for the elementwise gather
    scores_flat = bass.AP(tensor=scores.tensor, offset=0, ap=[[1, T * V], [1, 1]])

    sums_all = singles.tile([P, NBLK, NCHUNK], F32)   # per-(block, chunk) sum-exp
    idx_tile = singles.tile([P, NBLK], I32)
    starget = singles.tile([P, NBLK], F32)            # scores[t, target[t]]
    len_tile = singles.tile([P, 1], F32)
    nc.sync.dma_start(out=len_tile[:, :], in_=len_f)

    for j in range(NBLK):
        nc.sync.dma_start(out=idx_tile[:, j : j + 1], in_=gidx[j])
    for j in range(NBLK):
        nc.gpsimd.indirect_dma_start(
            out=starget[:, j : j + 1],
            out_offset=None,
            in_=scores_flat,
            in_offset=bass.IndirectOffsetOnAxis(ap=idx_tile[:, j : j + 1], axis=0),
        )

    # ---- main streaming pass: 64 tiles of [128, 4000] f32 (2 MB DMAs) ----
    # ACT computes exp in-place and accumulates the per-partition sum in the
    # same instruction, so DVE stays free and DMA is the only bottleneck.
    for j in range(NBLK):
        for c in range(NCHUNK):
            tl = data.tile([P, VC], F32)
            nc.sync.dma_start(
                out=tl[:, :],
                in_=scores[j * P : (j + 1) * P, c * VC : (c + 1) * VC],
            )
            nc.scalar.activation(
                out=tl[:, :],
                in_=tl[:, :],
                func=Act.Exp,
                accum_out=sums_all[:, j, c : c + 1],
            )

    # ---- per-t log-sum-exp and lp_t, in [P, NBLK] layout (t = j*128 + p) ----
    se = singles.tile([P, NBLK], F32)
    for j in range(NBLK):
        nc.vector.reduce_sum(
            out=se[:, j : j + 1], in_=sums_all[:, j, :], axis=mybir.AxisListType.X
        )
    lse = singles.tile([P, NBLK], F32)
    nc.scalar.activation(out=lse[:, :], in_=se[:, :], func=Act.Ln)

    iota_i = singles.tile([P, NBLK], I32)
    nc.gpsimd.iota(iota_i[:, :], pattern=[[P, NBLK]], base=0, channel_multiplier=1)
    iota_f = singles.tile([P, NBLK], F32)
    nc.vector.tensor_copy(iota_f[:, :], iota_i[:, :])
    mask8 = singles.tile([P, NBLK], F32)
    nc.vector.tensor_scalar(
        out=mask8[:, :], in0=iota_f[:, :], scalar1=len_tile[:, 0:1], scalar2=None,
        op0=Alu.is_lt,
    )

    # cols 0..7: lp_masked; cols 8..15: u = 0.7*exp(lp_masked) = exp(lp + ln 0.7)
    lpu = singles.tile([P, 2 * NBLK], F32)
    nc.vector.tensor_tensor(
        out=lpu[:, 0:NBLK], in0=starget[:, :], in1=lse[:, :], op=Alu.subtract
    )
    nc.vector.tensor_tensor(
        out=lpu[:, 0:NBLK], in0=lpu[:, 0:NBLK], in1=mask8[:, :], op=Alu.mult
    )
    ln07 = singles.tile([P, 1], F32)
    nc.vector.memset(ln07[:, :], LN07)
    nc.scalar.activation(
        out=lpu[:, NBLK : 2 * NBLK], in_=lpu[:, 0:NBLK], func=Act.Exp,
        bias=ln07[:, 0:1],
    )

    # ---- transpose [128, 16] -> [16, 128] and assemble [1, 1024] rows ----
    identity = singles.tile([P, P], F32)
    make_identity(nc, identity[:, :])
    pt = psum.tile([2 * NBLK, P], F32)
    nc.tensor.transpose(out=pt[:, :], in_=lpu[:, :], identity=identity[:, :])
    tails = singles.tile([2 * NBLK, P], F32)
    nc.vector.tensor_copy(tails[:, :], pt[:, :])

    lp_row = singles.tile([1, T], F32)
    u_row = singles.tile([1, T], F32)
    nc.sync.dma_start(
        out=lp_row[:, :].rearrange("a (b c) -> a b c", b=NBLK, c=P),
        in_=tails[0:NBLK, :],
    )
    nc.sync.dma_start(
        out=u_row[:, :].rearrange("a (b c) -> a b c", b=NBLK, c=P),
        in_=tails[NBLK : 2 * NBLK, :],
    )

    # ---- leaky integrator: props[t] = 0.3*props[t-1] + u[t-1], props[0]=0.5 ----
    c03 = singles.tile([1, T], F32)
    nc.vector.memset(c03[:, :], 0.3)
    props = singles.tile([1, T], F32)
    nc.vector.memset(props[0:1, 0:1], 0.5)
    nc.vector.tensor_tensor_scan(
        out=props[0:1, 1:T],
        data0=c03[0:1, 0 : T - 1],
        data1=u_row[0:1, 0 : T - 1],
        initial=0.5,
        op0=Alu.mult,
        op1=Alu.add,
    )

    # ---- ragged softmax over valid prefix, scaled by len; dot with lp ----
    iota_row_i = singles.tile([1, T], I32)
    nc.gpsimd.iota(iota_row_i[:, :], pattern=[[1, T]], base=0, channel_multiplier=0)
    iota_row_f = singles.tile([1, T], F32)
    nc.vector.tensor_copy(iota_row_f[:, :], iota_row_i[:, :])
    mask_row = singles.tile([1, T], F32)
    nc.vector.tensor_scalar(
        out=mask_row[:, :], in0=iota_row_f[:, :], scalar1=len_tile[0:1, 0:1],
        scalar2=None, op0=Alu.is_lt,
    )

    pm = singles.tile([1, T], F32)
    nc.vector.tensor_tensor(out=pm[:, :], in0=props[:, :], in1=mask_row[:, :], op=Alu.mult)
    m11 = singles.tile([1, 1], F32)
    nc.vector.reduce_max(out=m11[:, :], in_=pm[:, :], axis=mybir.AxisListType.X)
    negm = singles.tile([1, 1], F32)
    nc.vector.tensor_scalar_mul(out=negm[:, :], in0=m11[:, :], scalar1=-1.0)
    e_row = singles.tile([1, T], F32)
    nc.scalar.activation(
        out=e_row[:, :], in_=pm[:, :], func=Act.Exp, bias=negm[0:1, 0:1], scale=1.0
    )
    nc.vector.tensor_tensor(out=e_row[:, :], in0=e_row[:, :], in1=mask_row[:, :], op=Alu.mult)
    s11 = singles.tile([1, 1], F32)
    nc.vector.reduce_sum(out=s11[:, :], in_=e_row[:, :], axis=mybir.AxisListType.X)
    rs11 = singles.tile([1, 1], F32)
    nc.vector.reciprocal(out=rs11[:, :], in_=s11[:, :])
    f11 = singles.tile([1, 1], F32)
    nc.vector.tensor_tensor(out=f11[:, :], in0=rs11[:, :], in1=len_tile[0:1, 0:1], op=Alu.mult)

    prod = singles.tile([1, T], F32)
    nc.vector.tensor_tensor(out=prod[:, :], in0=lp_row[:, :], in1=e_row[:, :], op=Alu.mult)
    d11 = singles.tile([1, 1], F32)
    nc.vector.reduce_sum(out=d11[:, :], in_=prod[:, :], axis=mybir.AxisListType.X)
    o11 = singles.tile([1, 1], F32)
    nc.vector.tensor_tensor(out=o11[:, :], in0=d11[:, :], in1=f11[:, :], op=Alu.mult)
    nc.sync.dma_start(out=out, in_=o11[:, :])


_program_cache: dict[str, object] = {}


def build_program():
    if "nc" in _program_cache:
        return _program_cache["nc"]
    nc = bacc.Bacc(
        "TRN2", target_bir_lowering=False, debug=False, num_devices=N_CORES
    )
    scores = nc.dram_tensor("scores", [T, V], F32, kind="ExternalInput").ap()
    gidx = nc.dram_tensor("gidx", [NBLK, P, 1], I32, kind="ExternalInput").ap()
    len_f = nc.dram_tensor("len_f", [P, 1], F32, kind="ExternalInput").ap()
    out = nc.dram_tensor("out", [1, 1], F32, kind="ExternalOutput").ap()

    with tile.TileContext(nc) as tc, ExitStack() as ctx:
        _emit(ctx, tc, scores, gidx, len_f, out)
    nc.compile()
    _program_cache["nc"] = nc
    return nc


def make_in_maps(scores, target, lengths):
    scores = np.asarray(scores, dtype=np.float32)
    target = np.asarray(target).astype(np.int64)
    lengths = np.asarray(lengths).astype(np.int64)
    t_base = np.arange(T, dtype=np.int64) * V
    in_maps = []
    for b in range(B):
        g = (t_base + target[b]).astype(np.int32).reshape(NBLK, P, 1)
        in_maps.append(
            {
                "scores": np.ascontiguousarray(scores[b]),
                "gidx": g,
                "len_f": np.full((P, 1), float(lengths[b]), dtype=np.float32),
            }
        )
    return in_maps


def finish(partials, lengths):
    lengths = np.asarray(lengths).astype(np.int64)
    total = float(lengths.sum())
    return np.float32(-float(np.sum(partials)) / total)


def kernel(scores, target, lengths, _trace: bool = False):
    nc = build_program()
    in_maps = make_in_maps(scores, target, lengths)
    res = run_bass_kernel_spmd(nc, in_maps, core_ids=list(range(N_CORES)), trace=_trace)
    partials = [float(res.results[i]["out"][0, 0]) for i in range(N_CORES)]
    loss = finish(partials, lengths)
    if _trace:
        kernel.last_results = res
    return loss
